# revision 25
# baseline (speedup 1.0000x reference)
"""Kronecker layer forward on 8 TRN2 NeuronCores.

Computes y = gelu_exact(x @ kron(B, A)) + bias for
  x [16384, 4096] f32, A [64, 64], B [64, 64], bias [4096].

Math: with x3 = x.reshape(n, 64, 64) (feature f = i*64 + k),
  y[b, j*64+l] = sum_{i,k} x3[b,i,k] * B[i,j] * A[k,l].

Per supertile s we pick 4 tokens t(g,h) = g*(tpc/2) + h*(tpc/4) + s
(g,h in {0,1}) and form one 128x128 SBUF tile
  xt[(g,i), (h,k)] = x[t(g,h), i*64+k]
then chain two TensorE matmuls with the DATA as the stationary operand and
block-diagonal factors as the moving one:
  o1 = xt.T @ blockdiag(B,B)   -> [(h,k),(g,j)]
  o2 = u.T  @ blockdiag(A,A)   -> [(g,j),(h,l)]
(u = o1 copied to SBUF). o2 is exactly the y-layout view
y[t(g,h), j*64+l] - both contraction dims land on partitions with zero
transposes.

The whole pipeline runs in bf16 (inputs quantized on the host, outputs
returned as bf16 and upcast on the host): the problem is DMA-bound, so
halving the bytes halves the bottleneck, and bf16 matmuls run at 1
cycle/row at any moving width (f32r needs a 256-wide moving operand and
burns half of it as garbage). Measured end-to-end rel err ~4e-3.

Streaming schedule: tokens are processed in blocks; each block is one
contiguous p-major input DMA ([128, nb*128] slab) on the sync HWDGE
queue and two output half-slabs on gpsimd's SWDGE queue (measured
fastest queue arrangement - HWDGE reads + SWDGE writes don't contend).
The block sizes are non-uniform: small head blocks so the first matmul
starts ~6us into the run (instead of waiting ~14us for a 1MB block to
land during DMA ramp-up), and small tail blocks so the last output
slab drains quickly after the final gelu.

Sharding: pure data-parallel over the token dim - 2048 tokens per core,
A/B/bias replicated, no collectives.
"""

import numpy as np

N_CORES = 8
TOKENS = 16384
D = 4096
TPC = TOKENS // N_CORES  # tokens per core

_CACHE = {}

GRP = 8  # supertiles per PSUM pack (2 banks per tile)


def _block_sizes(nsuper):
    """Supertiles per block: small head blocks (fast pipeline start),
    big middle blocks (DMA efficiency), small tail blocks (fast drain)."""
    if nsuper >= 64:
        sizes = [8, 24]
        rem = nsuper - 64
        assert rem % 32 == 0
        sizes += [32] * (rem // 32)
        sizes += [16, 16]
    else:
        nb = min(16, nsuper)
        assert nsuper % nb == 0
        sizes = [nb] * (nsuper // nb)
    assert sum(sizes) == nsuper and all(s % GRP == 0 for s in sizes)
    return sizes


def _build_bf16(tpc, with_bias, n_cores):
    import concourse.bacc as bacc
    import concourse.mybir as mybir
    import concourse.tile as tile

    f32 = mybir.dt.float32
    bf16 = mybir.dt.bfloat16

    nsuper = tpc // 4
    sizes = _block_sizes(nsuper)
    nbmax = max(sizes)

    nc = bacc.Bacc(
        "TRN2",
        target_bir_lowering=False,
        debug=False,
        num_devices=n_cores,
    )
    x_slabs = [
        nc.dram_tensor(f"x{i}", [128, nb * 128], bf16, kind="ExternalInput").ap()
        for i, nb in enumerate(sizes)
    ]
    y_slabs = [
        nc.dram_tensor(f"y{i}", [2, 64, nb * 128], bf16, kind="ExternalOutput").ap()
        for i, nb in enumerate(sizes)
    ]
    b_d = nc.dram_tensor("bd", [128, 128], bf16, kind="ExternalInput").ap()
    a_d = nc.dram_tensor("ad", [128, 128], bf16, kind="ExternalInput").ap()
    if with_bias:
        bias_d = nc.dram_tensor("bias_t", [128, 128], bf16, kind="ExternalInput").ap()

    with tile.TileContext(nc) as tc:
        with (
            tc.tile_pool(name="const", bufs=1) as constp,
            tc.tile_pool(name="xp", bufs=4) as xp,
            tc.tile_pool(name="up", bufs=4) as up,
            tc.tile_pool(name="yp", bufs=4) as yp,
            tc.tile_pool(name="ps1", bufs=2, space="PSUM") as ps1,
            tc.tile_pool(name="ps2", bufs=2, space="PSUM") as ps2,
        ):
            bd = constp.tile([128, 128], bf16)
            nc.sync.dma_start(bd[:], b_d)
            ad = constp.tile([128, 128], bf16)
            nc.sync.dma_start(ad[:], a_d)
            if with_bias:
                bias_t = constp.tile([128, 128], bf16)
                nc.sync.dma_start(bias_t[:], bias_d)

            # Warm DVE and Act while the first x block is still in
            # flight; in particular the Gelu table load (~1.3us) must
            # not land in block 0's critical path. SBUF-only sources
            # (PSUM is exactly full with the ps1/ps2 pools).
            wsb = up.tile([128, GRP * 128], bf16)
            nc.vector.tensor_copy(wsb[:, :128], bd[:])
            nc.scalar.activation(
                wsb[:, 128:256], bd[:],
                mybir.ActivationFunctionType.Gelu,
            )

            for i, nb in enumerate(sizes):
                xbig = xp.tile([128, nbmax * 128], bf16)
                ybig = yp.tile([128, nbmax * 128], bf16)
                nc.sync.dma_start(xbig[:, : nb * 128], x_slabs[i])

                for grp in range(nb // GRP):
                    o1 = ps1.tile([128, GRP * 128], f32)
                    o2 = ps2.tile([128, GRP * 128], f32)
                    u = up.tile([128, GRP * 128], bf16)
                    for q in range(GRP):
                        s = grp * GRP + q
                        nc.tensor.matmul(
                            o1[:, q * 128 : (q + 1) * 128],
                            xbig[:, s * 128 : (s + 1) * 128],
                            bd[:],
                        )
                    nc.vector.tensor_copy(
                        u[:].rearrange("p (q f) -> p q f", f=128),
                        o1[:].rearrange("p (q f) -> p q f", f=128),
                    )
                    for q in range(GRP):
                        nc.tensor.matmul(
                            o2[:, q * 128 : (q + 1) * 128],
                            u[:, q * 128 : (q + 1) * 128],
                            ad[:],
                        )
                    W = GRP * 128
                    ydst = ybig[:, grp * W : (grp + 1) * W].rearrange(
                        "p (q f) -> p q f", f=128
                    )
                    nc.scalar.activation(
                        ydst,
                        o2[:].rearrange("p (q f) -> p q f", f=128),
                        mybir.ActivationFunctionType.Gelu,
                    )
                    if with_bias:
                        bseg = ybig[:, grp * W : (grp + 1) * W].rearrange(
                            "p (q f) -> p q f", f=128
                        )
                        bsrc = bias_t[:].unsqueeze(1).broadcast_to([128, GRP, 128])
                        nc.vector.tensor_add(bseg, bseg, bsrc)

                nc.gpsimd.dma_start(y_slabs[i][0], ybig[:64, : nb * 128])
                nc.scalar.dma_start(y_slabs[i][1], ybig[64:, : nb * 128])

    nc.compile()
    return nc


def _get_nc(tpc, mm_impl, with_bias, n_cores=N_CORES):
    key = (tpc, mm_impl, with_bias, n_cores)
    if key not in _CACHE:
        assert mm_impl == "bf16"
        _CACHE[key] = _build_bf16(tpc, with_bias, n_cores)
    return _CACHE[key]


def _np_bf16():
    import ml_dtypes

    return ml_dtypes.bfloat16


def _make_weights(A, B):
    dt = _np_bf16()
    Bd = np.zeros((128, 128), np.float32)
    Bd[:64, :64] = B
    Bd[64:, 64:] = B
    Ad = np.zeros((128, 128), np.float32)
    Ad[:64, :64] = A
    Ad[64:, 64:] = A
    return {"bd": Bd.astype(dt), "ad": Ad.astype(dt)}


def _run(x, A, B, bias, mm_impl="bf16", tpc=TPC, trace=False):
    from concourse.bass_utils import run_bass_kernel_spmd

    n = x.shape[0]
    n_cores = n // tpc
    assert n == n_cores * tpc

    with_bias = bool(np.any(bias))
    nc = _get_nc(tpc, mm_impl, with_bias, n_cores)
    wmaps = _make_weights(np.asarray(A, np.float32), np.asarray(B, np.float32))

    nsuper = tpc // 4
    sizes = _block_sizes(nsuper)
    offs = np.cumsum([0] + sizes)[:-1]  # supertile offset per block
    dt = _np_bf16()
    half = tpc // 2

    def slab_maps(xs):
        # xs [tpc, D] f32 -> per-block p-major slabs [128, nb*128] bf16
        # P[g, i, r, k] = x[g*half + r, i*64 + k]
        P = xs.astype(dt).reshape(2, half, 64, 64).transpose(0, 2, 1, 3)
        m = {}
        for i, (s0, nb) in enumerate(zip(offs, sizes)):
            v = P[:, :, 2 * s0 : 2 * (s0 + nb), :]  # [2, 64, 2nb, 64]
            m[f"x{i}"] = np.ascontiguousarray(v).reshape(128, nb * 128)
        return m

    def unslab(rd):
        # per-block [2, 64, nb*128] bf16 -> [tpc, D] f32
        Y = np.empty((2, 64, half, 64), np.float32)
        for i, (s0, nb) in enumerate(zip(offs, sizes)):
            v = np.asarray(rd[f"y{i}"]).astype(np.float32)
            Y[:, :, 2 * s0 : 2 * (s0 + nb), :] = v.reshape(2, 64, 2 * nb, 64)
        return Y.transpose(0, 2, 1, 3).reshape(tpc, D)

    in_maps = []
    for c in range(n_cores):
        m = slab_maps(np.asarray(x[c * tpc : (c + 1) * tpc], dtype=np.float32))
        m.update(wmaps)
        if with_bias:
            m["bias_t"] = np.ascontiguousarray(
                np.tile(bias.astype(np.float32).reshape(64, 64), (2, 2))
            ).astype(dt)
        in_maps.append(m)

    res = run_bass_kernel_spmd(
        nc, in_maps, list(range(n_cores)), trace=trace,
        trace_cores=list(range(n_cores)) if trace else None,
    )
    y = np.concatenate([unslab(r) for r in res.results], axis=0)
    return y.astype(np.float32), res


def kernel(x, A, B, bias):
    y, _ = _run(
        np.asarray(x), np.asarray(A), np.asarray(B), np.asarray(bias),
        mm_impl="bf16",
    )
    return y


# revision 26
# speedup vs baseline: 1.0350x; 1.0350x over previous
"""Kronecker layer forward on 8 TRN2 NeuronCores.

Computes y = gelu_exact(x @ kron(B, A)) + bias for
  x [16384, 4096] f32, A [64, 64], B [64, 64], bias [4096].

Math: with x3 = x.reshape(n, 64, 64) (feature f = i*64 + k),
  y[b, j*64+l] = sum_{i,k} x3[b,i,k] * B[i,j] * A[k,l].

Per supertile s we pick 4 tokens t(g,h) = g*(tpc/2) + h*(tpc/4) + s
(g,h in {0,1}) and form one 128x128 SBUF tile
  xt[(g,i), (h,k)] = x[t(g,h), i*64+k]
then chain two TensorE matmuls with the DATA as the stationary operand and
block-diagonal factors as the moving one:
  o1 = xt.T @ blockdiag(B,B)   -> [(h,k),(g,j)]
  o2 = u.T  @ blockdiag(A,A)   -> [(g,j),(h,l)]
(u = o1 copied to SBUF). o2 is exactly the y-layout view
y[t(g,h), j*64+l] - both contraction dims land on partitions with zero
transposes.

The whole pipeline runs in bf16 (inputs quantized on the host, outputs
returned as bf16 and upcast on the host): the problem is DMA-bound, so
halving the bytes halves the bottleneck, and bf16 matmuls run at 1
cycle/row at any moving width (f32r needs a 256-wide moving operand and
burns half of it as garbage). Measured end-to-end rel err ~4e-3.

Streaming schedule: tokens are processed in blocks; each block is one
contiguous p-major input DMA ([128, nb*128] slab) on the sync HWDGE
queue and two output half-slabs on gpsimd's SWDGE queue (measured
fastest queue arrangement - HWDGE reads + SWDGE writes don't contend).
The block sizes are non-uniform: small head blocks so the first matmul
starts ~6us into the run (instead of waiting ~14us for a 1MB block to
land during DMA ramp-up), and small tail blocks so the last output
slab drains quickly after the final gelu.

Sharding: pure data-parallel over the token dim - 2048 tokens per core,
A/B/bias replicated, no collectives.
"""

import numpy as np

N_CORES = 8
TOKENS = 16384
D = 4096
TPC = TOKENS // N_CORES  # tokens per core

_CACHE = {}

GRP = 8  # supertiles per PSUM pack (2 banks per tile)


def _block_sizes(nsuper):
    """Supertiles per block: small head blocks (fast pipeline start),
    big middle blocks (DMA efficiency), small tail blocks (fast drain)."""
    if nsuper >= 64:
        sizes = [8, 24]
        rem = nsuper - 64
        assert rem % 32 == 0
        sizes += [32] * (rem // 32)
        sizes += [16, 16]
    else:
        nb = min(16, nsuper)
        assert nsuper % nb == 0
        sizes = [nb] * (nsuper // nb)
    assert sum(sizes) == nsuper and all(s % GRP == 0 for s in sizes)
    return sizes


def _build_bf16(tpc, with_bias, n_cores):
    import concourse.bacc as bacc
    import concourse.mybir as mybir
    import concourse.tile as tile

    f32 = mybir.dt.float32
    bf16 = mybir.dt.bfloat16

    nsuper = tpc // 4
    sizes = _block_sizes(nsuper)
    nbmax = max(sizes)

    nc = bacc.Bacc(
        "TRN2",
        target_bir_lowering=False,
        debug=False,
        num_devices=n_cores,
    )
    x_slabs = [
        nc.dram_tensor(f"x{i}", [128, nb * 128], bf16, kind="ExternalInput").ap()
        for i, nb in enumerate(sizes)
    ]
    y_slabs = [
        nc.dram_tensor(f"y{i}", [128, nb * 128], bf16, kind="ExternalOutput").ap()
        for i, nb in enumerate(sizes)
    ]
    b_d = nc.dram_tensor("bd", [128, 128], bf16, kind="ExternalInput").ap()
    a_d = nc.dram_tensor("ad", [128, 128], bf16, kind="ExternalInput").ap()
    if with_bias:
        bias_d = nc.dram_tensor("bias_t", [128, 128], bf16, kind="ExternalInput").ap()

    with tile.TileContext(nc) as tc:
        with (
            tc.tile_pool(name="const", bufs=1) as constp,
            tc.tile_pool(name="xp", bufs=4) as xp,
            tc.tile_pool(name="up", bufs=4) as up,
            tc.tile_pool(name="yp", bufs=4) as yp,
            tc.tile_pool(name="ps1", bufs=2, space="PSUM") as ps1,
            tc.tile_pool(name="ps2", bufs=2, space="PSUM") as ps2,
        ):
            bd = constp.tile([128, 128], bf16)
            nc.sync.dma_start(bd[:], b_d)
            ad = constp.tile([128, 128], bf16)
            nc.sync.dma_start(ad[:], a_d)
            if with_bias:
                bias_t = constp.tile([128, 128], bf16)
                nc.sync.dma_start(bias_t[:], bias_d)

            # Warm DVE and Act while the first x block is still in
            # flight; in particular the Gelu table load (~1.3us) must
            # not land in block 0's critical path. SBUF-only sources
            # (PSUM is exactly full with the ps1/ps2 pools).
            wsb = up.tile([128, GRP * 128], bf16)
            nc.vector.tensor_copy(wsb[:, :128], bd[:])
            nc.scalar.activation(
                wsb[:, 128:256], bd[:],
                mybir.ActivationFunctionType.Gelu,
            )

            for i, nb in enumerate(sizes):
                xbig = xp.tile([128, nbmax * 128], bf16)
                ybig = yp.tile([128, nbmax * 128], bf16)
                nc.sync.dma_start(xbig[:, : nb * 128], x_slabs[i])

                for grp in range(nb // GRP):
                    o1 = ps1.tile([128, GRP * 128], f32)
                    o2 = ps2.tile([128, GRP * 128], f32)
                    u = up.tile([128, GRP * 128], bf16)
                    for q in range(GRP):
                        s = grp * GRP + q
                        nc.tensor.matmul(
                            o1[:, q * 128 : (q + 1) * 128],
                            xbig[:, s * 128 : (s + 1) * 128],
                            bd[:],
                        )
                    nc.vector.tensor_copy(
                        u[:].rearrange("p (q f) -> p q f", f=128),
                        o1[:].rearrange("p (q f) -> p q f", f=128),
                    )
                    for q in range(GRP):
                        nc.tensor.matmul(
                            o2[:, q * 128 : (q + 1) * 128],
                            u[:, q * 128 : (q + 1) * 128],
                            ad[:],
                        )
                    W = GRP * 128
                    ydst = ybig[:, grp * W : (grp + 1) * W].rearrange(
                        "p (q f) -> p q f", f=128
                    )
                    nc.scalar.activation(
                        ydst,
                        o2[:].rearrange("p (q f) -> p q f", f=128),
                        mybir.ActivationFunctionType.Gelu,
                    )
                    if with_bias:
                        bseg = ybig[:, grp * W : (grp + 1) * W].rearrange(
                            "p (q f) -> p q f", f=128
                        )
                        bsrc = bias_t[:].unsqueeze(1).broadcast_to([128, GRP, 128])
                        nc.vector.tensor_add(bseg, bseg, bsrc)

                nc.gpsimd.dma_start(y_slabs[i], ybig[:, : nb * 128])

    nc.compile()
    return nc


def _get_nc(tpc, mm_impl, with_bias, n_cores=N_CORES):
    key = (tpc, mm_impl, with_bias, n_cores)
    if key not in _CACHE:
        assert mm_impl == "bf16"
        _CACHE[key] = _build_bf16(tpc, with_bias, n_cores)
    return _CACHE[key]


def _np_bf16():
    import ml_dtypes

    return ml_dtypes.bfloat16


def _make_weights(A, B):
    dt = _np_bf16()
    Bd = np.zeros((128, 128), np.float32)
    Bd[:64, :64] = B
    Bd[64:, 64:] = B
    Ad = np.zeros((128, 128), np.float32)
    Ad[:64, :64] = A
    Ad[64:, 64:] = A
    return {"bd": Bd.astype(dt), "ad": Ad.astype(dt)}


def _run(x, A, B, bias, mm_impl="bf16", tpc=TPC, trace=False):
    from concourse.bass_utils import run_bass_kernel_spmd

    n = x.shape[0]
    n_cores = n // tpc
    assert n == n_cores * tpc

    with_bias = bool(np.any(bias))
    nc = _get_nc(tpc, mm_impl, with_bias, n_cores)
    wmaps = _make_weights(np.asarray(A, np.float32), np.asarray(B, np.float32))

    nsuper = tpc // 4
    sizes = _block_sizes(nsuper)
    offs = np.cumsum([0] + sizes)[:-1]  # supertile offset per block
    dt = _np_bf16()
    half = tpc // 2

    def slab_maps(xs):
        # xs [tpc, D] f32 -> per-block p-major slabs [128, nb*128] bf16
        # P[g, i, r, k] = x[g*half + r, i*64 + k]
        P = xs.astype(dt).reshape(2, half, 64, 64).transpose(0, 2, 1, 3)
        m = {}
        for i, (s0, nb) in enumerate(zip(offs, sizes)):
            v = P[:, :, 2 * s0 : 2 * (s0 + nb), :]  # [2, 64, 2nb, 64]
            m[f"x{i}"] = np.ascontiguousarray(v).reshape(128, nb * 128)
        return m

    def unslab(rd):
        # per-block [2, 64, nb*128] bf16 -> [tpc, D] f32
        Y = np.empty((2, 64, half, 64), np.float32)
        for i, (s0, nb) in enumerate(zip(offs, sizes)):
            v = np.asarray(rd[f"y{i}"]).astype(np.float32)
            Y[:, :, 2 * s0 : 2 * (s0 + nb), :] = v.reshape(2, 64, 2 * nb, 64)
        return Y.transpose(0, 2, 1, 3).reshape(tpc, D)

    in_maps = []
    for c in range(n_cores):
        m = slab_maps(np.asarray(x[c * tpc : (c + 1) * tpc], dtype=np.float32))
        m.update(wmaps)
        if with_bias:
            m["bias_t"] = np.ascontiguousarray(
                np.tile(bias.astype(np.float32).reshape(64, 64), (2, 2))
            ).astype(dt)
        in_maps.append(m)

    res = run_bass_kernel_spmd(
        nc, in_maps, list(range(n_cores)), trace=trace,
        trace_cores=list(range(n_cores)) if trace else None,
    )
    y = np.concatenate([unslab(r) for r in res.results], axis=0)
    return y.astype(np.float32), res


def kernel(x, A, B, bias):
    y, _ = _run(
        np.asarray(x), np.asarray(A), np.asarray(B), np.asarray(bias),
        mm_impl="bf16",
    )
    return y


# revision 27
# speedup vs baseline: 1.0607x; 1.0248x over previous
"""Kronecker layer forward on 8 TRN2 NeuronCores.

Computes y = gelu_exact(x @ kron(B, A)) + bias for
  x [16384, 4096] f32, A [64, 64], B [64, 64], bias [4096].

Math: with x3 = x.reshape(n, 64, 64) (feature f = i*64 + k),
  y[b, j*64+l] = sum_{i,k} x3[b,i,k] * B[i,j] * A[k,l].

Per supertile s we pick 4 tokens t(g,h) = g*(tpc/2) + h*(tpc/4) + s
(g,h in {0,1}) and form one 128x128 SBUF tile
  xt[(g,i), (h,k)] = x[t(g,h), i*64+k]
then chain two TensorE matmuls with the DATA as the stationary operand and
block-diagonal factors as the moving one:
  o1 = xt.T @ blockdiag(B,B)   -> [(h,k),(g,j)]
  o2 = u.T  @ blockdiag(A,A)   -> [(g,j),(h,l)]
(u = o1 copied to SBUF). o2 is exactly the y-layout view
y[t(g,h), j*64+l] - both contraction dims land on partitions with zero
transposes.

The whole pipeline runs in bf16 (inputs quantized on the host, outputs
returned as bf16 and upcast on the host): the problem is DMA-bound, so
halving the bytes halves the bottleneck, and bf16 matmuls run at 1
cycle/row at any moving width (f32r needs a 256-wide moving operand and
burns half of it as garbage). Measured end-to-end rel err ~4e-3.

Streaming schedule: tokens are processed in blocks; each block is one
contiguous p-major input DMA ([128, nb*128] slab) on the sync HWDGE
queue and two output half-slabs on gpsimd's SWDGE queue (measured
fastest queue arrangement - HWDGE reads + SWDGE writes don't contend).
The block sizes are non-uniform: small head blocks so the first matmul
starts ~6us into the run (instead of waiting ~14us for a 1MB block to
land during DMA ramp-up), and small tail blocks so the last output
slab drains quickly after the final gelu.

Sharding: pure data-parallel over the token dim - 2048 tokens per core,
A/B/bias replicated, no collectives.
"""

import numpy as np

N_CORES = 8
TOKENS = 16384
D = 4096
TPC = TOKENS // N_CORES  # tokens per core

_CACHE = {}

GRP = 8  # supertiles per PSUM pack (2 banks per tile)


def _block_sizes(nsuper):
    """Supertiles per block: small head blocks (fast pipeline start),
    big middle blocks (DMA efficiency), small tail blocks (fast drain)."""
    if nsuper >= 64:
        sizes = [8, 24]
        rem = nsuper - 64
        assert rem % 32 == 0
        sizes += [32] * (rem // 32)
        sizes += [16, 16]
    else:
        nb = min(16, nsuper)
        assert nsuper % nb == 0
        sizes = [nb] * (nsuper // nb)
    assert sum(sizes) == nsuper and all(s % GRP == 0 for s in sizes)
    return sizes


def _build_bf16(tpc, with_bias, n_cores):
    import concourse.bacc as bacc
    import concourse.mybir as mybir
    import concourse.tile as tile

    f32 = mybir.dt.float32
    bf16 = mybir.dt.bfloat16

    nsuper = tpc // 4
    sizes = _block_sizes(nsuper)
    nbmax = max(sizes)

    nc = bacc.Bacc(
        "TRN2",
        target_bir_lowering=False,
        debug=False,
        num_devices=n_cores,
    )
    x_slabs = [
        nc.dram_tensor(f"x{i}", [128, nb * 128], bf16, kind="ExternalInput").ap()
        for i, nb in enumerate(sizes)
    ]
    y_slabs = [
        nc.dram_tensor(f"y{i}", [128, nb * 128], bf16, kind="ExternalOutput").ap()
        for i, nb in enumerate(sizes)
    ]
    b_d = nc.dram_tensor("bd", [128, 128], bf16, kind="ExternalInput").ap()
    a_d = nc.dram_tensor("ad", [128, 128], bf16, kind="ExternalInput").ap()
    if with_bias:
        bias_d = nc.dram_tensor("bias_t", [128, 128], bf16, kind="ExternalInput").ap()

    with tile.TileContext(nc) as tc:
        with (
            tc.tile_pool(name="const", bufs=1) as constp,
            tc.tile_pool(name="xp", bufs=4) as xp,
            tc.tile_pool(name="up", bufs=4) as up,
            tc.tile_pool(name="yp", bufs=4) as yp,
            tc.tile_pool(name="ps1", bufs=2, space="PSUM") as ps1,
            tc.tile_pool(name="ps2", bufs=2, space="PSUM") as ps2,
        ):
            bd = constp.tile([128, 128], bf16)
            nc.sync.dma_start(bd[:], b_d)
            ad = constp.tile([128, 128], bf16)
            nc.sync.dma_start(ad[:], a_d)
            if with_bias:
                bias_t = constp.tile([128, 128], bf16)
                nc.sync.dma_start(bias_t[:], bias_d)

            for i, nb in enumerate(sizes):
                xbig = xp.tile([128, nbmax * 128], bf16)
                ybig = yp.tile([128, nbmax * 128], bf16)
                nc.sync.dma_start(xbig[:, : nb * 128], x_slabs[i])

                for grp in range(nb // GRP):
                    o1 = ps1.tile([128, GRP * 128], f32)
                    o2 = ps2.tile([128, GRP * 128], f32)
                    u = up.tile([128, GRP * 128], bf16)
                    for q in range(GRP):
                        s = grp * GRP + q
                        nc.tensor.matmul(
                            o1[:, q * 128 : (q + 1) * 128],
                            xbig[:, s * 128 : (s + 1) * 128],
                            bd[:],
                        )
                    nc.vector.tensor_copy(
                        u[:].rearrange("p (q f) -> p q f", f=128),
                        o1[:].rearrange("p (q f) -> p q f", f=128),
                    )
                    for q in range(GRP):
                        nc.tensor.matmul(
                            o2[:, q * 128 : (q + 1) * 128],
                            u[:, q * 128 : (q + 1) * 128],
                            ad[:],
                        )
                    W = GRP * 128
                    ydst = ybig[:, grp * W : (grp + 1) * W].rearrange(
                        "p (q f) -> p q f", f=128
                    )
                    nc.scalar.activation(
                        ydst,
                        o2[:].rearrange("p (q f) -> p q f", f=128),
                        mybir.ActivationFunctionType.Gelu,
                    )
                    if with_bias:
                        bseg = ybig[:, grp * W : (grp + 1) * W].rearrange(
                            "p (q f) -> p q f", f=128
                        )
                        bsrc = bias_t[:].unsqueeze(1).broadcast_to([128, GRP, 128])
                        nc.vector.tensor_add(bseg, bseg, bsrc)

                nc.gpsimd.dma_start(y_slabs[i], ybig[:, : nb * 128])

    nc.compile()
    return nc


def _get_nc(tpc, mm_impl, with_bias, n_cores=N_CORES):
    key = (tpc, mm_impl, with_bias, n_cores)
    if key not in _CACHE:
        assert mm_impl == "bf16"
        _CACHE[key] = _build_bf16(tpc, with_bias, n_cores)
    return _CACHE[key]


def _np_bf16():
    import ml_dtypes

    return ml_dtypes.bfloat16


def _make_weights(A, B):
    dt = _np_bf16()
    Bd = np.zeros((128, 128), np.float32)
    Bd[:64, :64] = B
    Bd[64:, 64:] = B
    Ad = np.zeros((128, 128), np.float32)
    Ad[:64, :64] = A
    Ad[64:, 64:] = A
    return {"bd": Bd.astype(dt), "ad": Ad.astype(dt)}


def _run(x, A, B, bias, mm_impl="bf16", tpc=TPC, trace=False):
    from concourse.bass_utils import run_bass_kernel_spmd

    n = x.shape[0]
    n_cores = n // tpc
    assert n == n_cores * tpc

    with_bias = bool(np.any(bias))
    nc = _get_nc(tpc, mm_impl, with_bias, n_cores)
    wmaps = _make_weights(np.asarray(A, np.float32), np.asarray(B, np.float32))

    nsuper = tpc // 4
    sizes = _block_sizes(nsuper)
    offs = np.cumsum([0] + sizes)[:-1]  # supertile offset per block
    dt = _np_bf16()
    half = tpc // 2

    def slab_maps(xs):
        # xs [tpc, D] f32 -> per-block p-major slabs [128, nb*128] bf16
        # P[g, i, r, k] = x[g*half + r, i*64 + k]
        P = xs.astype(dt).reshape(2, half, 64, 64).transpose(0, 2, 1, 3)
        m = {}
        for i, (s0, nb) in enumerate(zip(offs, sizes)):
            v = P[:, :, 2 * s0 : 2 * (s0 + nb), :]  # [2, 64, 2nb, 64]
            m[f"x{i}"] = np.ascontiguousarray(v).reshape(128, nb * 128)
        return m

    def unslab(rd):
        # per-block [2, 64, nb*128] bf16 -> [tpc, D] f32
        Y = np.empty((2, 64, half, 64), np.float32)
        for i, (s0, nb) in enumerate(zip(offs, sizes)):
            v = np.asarray(rd[f"y{i}"]).astype(np.float32)
            Y[:, :, 2 * s0 : 2 * (s0 + nb), :] = v.reshape(2, 64, 2 * nb, 64)
        return Y.transpose(0, 2, 1, 3).reshape(tpc, D)

    in_maps = []
    for c in range(n_cores):
        m = slab_maps(np.asarray(x[c * tpc : (c + 1) * tpc], dtype=np.float32))
        m.update(wmaps)
        if with_bias:
            m["bias_t"] = np.ascontiguousarray(
                np.tile(bias.astype(np.float32).reshape(64, 64), (2, 2))
            ).astype(dt)
        in_maps.append(m)

    res = run_bass_kernel_spmd(
        nc, in_maps, list(range(n_cores)), trace=trace,
        trace_cores=list(range(n_cores)) if trace else None,
    )
    y = np.concatenate([unslab(r) for r in res.results], axis=0)
    return y.astype(np.float32), res


def kernel(x, A, B, bias):
    y, _ = _run(
        np.asarray(x), np.asarray(A), np.asarray(B), np.asarray(bias),
        mm_impl="bf16",
    )
    return y
